# revision 43
# baseline (speedup 1.0000x reference)
"""CondConv2d on 8 Trainium2 NeuronCores — data-parallel over batch N=8.

Per-core (one sample), all conv data in bf16:
  - x is read from HBM ONCE (lower partitions 0-63) in 8 chunks; the row-
    shifted upper copy (partitions 64-127) is produced by SBUF->SBUF DMAs
    whose descriptors drain behind the remaining loads — the conv consumes
    upper chunks far later than they arrive.
  - The attention branch (three global-mean-pooled conv3ds) collapses to a
    linear function of basis sums of x.  The expensive per-channel totals
    exploit that the conv3d depth masks only differentiate channels
    {0,1,62,63}: a selector matmul accumulates [4 edge-channel totals +
    grand total] into one PSUM bank on the otherwise-idle PE as chunks land,
    and one 512-wide DVE reduce + per-partition coefficients absorb them.
    Edge rows/cols/corners and the trailing total-folds are tiny chunk-gated
    DVE/ACT ops writing columns of one [64,19] basis matrix; a fused 4-op
    DVE contraction produces the logits.
  - Softmax normalization is skipped: weights are mixed with raw exp(logits)
    (the static conv_w bank is pre-folded into each bank host-side since
    sum(att)=1), and the 1/sum(exp) scale is applied at PSUM eviction
    together with the conv bias.
  - The 3x3 conv runs even/odd-row interleaved at 75% PE utilization:
    PSUM partitions = 64 out-ch x {even,odd} rows, contraction 128 =
    64 in-ch x {row r, row r+1} (the shifted upper copy).  6 matmuls
    (2 row-bases x 3 width taps) of [128x128x390] cover SIX output rows
    (3 row-pairs) per PSUM tile — 22 tiles total, half the PE cycles of a
    64-wide layout.  The rhs is a strided (pair, 130) view of XL; output
    rows deinterleave in the two eviction DMAs.  Output is stored bf16.
  - att-gated filler matmuls bridge the softmax/mixing window so the PE
    p-state stays ramped into the conv.
"""
import numpy as np

CONV_DT = "bf16"
N, C, H, W = 8, 64, 128, 128
K = 4
WP = W + 2                 # padded row width (130)
NELEM = WP * H + 262       # per-partition x buffer length (16902)
JT = 3                     # row-pairs per conv tile (6 output rows)
NPAIR = H // 2
GS = 5                     # conv tiles in the m-outer head group
NPSB = 6                   # conv PSUM tile buffers
NT = (NPAIR + JT - 1) // JT  # 22

# fp8 attention copy of x: 4 load chunks (row boundaries gate the totals)
XCHUNK_ROWS = [32, 32, 32, 32]
XCHUNK_OFF = list(np.cumsum([0] + XCHUNK_ROWS))
# bf16 conv copy of x: chunks sized to stay ahead of the conv wavefront
BCHUNK_ROWS = [24, 24, 32, 32, 16]
BCHUNK_OFF = list(np.cumsum([0] + BCHUNK_ROWS))

# conv matmul configs: (row base b, width tap dw)
BCFG = [(-1, -1), (-1, 0), (-1, 1), (1, -1), (1, 0), (1, 1)]

NBASIS = 21
SMM_W = 512                # selector-matmul free width
NSMM = 12                  # selector matmuls cover elems [0, 6144)
SMM_END = SMM_W * NSMM
# DVE fold spans for the rest of the totals (gating is automatic)
FOLD_SPANS = [(6144, 8192), (8192, 10400), (13520, 14560), (16120, 16640)]
# spans summed on the ACT engine (basis cols 18..20)
ACT_SPANS = [(10400, 12480), (12480, 13520), (14560, 16120)]


# ----------------------------------------------------------------------------
# host-side prep
# ----------------------------------------------------------------------------
def _make_cw(net0_w, net0_b, net1_w, net1_b, net2_w, net2_b):
    """CW[c, b, k] over the 10 logical bases:
    0=total, 1=row0, 2=row127, 3=col0, 4=col127,
    5..8=corners (00,0W,H0,HW), 9=const 1."""
    cw = np.zeros((C, 10, K), np.float64)
    scale = 1.0 / (C * H * W)
    for w_net, pads in ((net0_w, (0, 0, 0)), (net1_w, (1, 1, 1)), (net2_w, (2, 1, 1))):
        Kk, _, kd, kh, kw = w_net.shape
        pd, ph, pw = pads
        for i in range(kd):
            clo, chi = max(0, i - pd), min(C - 1, C - 1 + i - pd)
            cmask = np.zeros(C)
            cmask[clo:chi + 1] = 1.0
            for j in range(kh):
                hlo, hhi = max(0, j - ph), min(H - 1, H - 1 + j - ph)
                dropA = 0 if hlo == 1 else (127 if hhi == H - 2 else None)
                for l in range(kw):
                    wlo, whi = max(0, l - pw), min(W - 1, W - 1 + l - pw)
                    dropB = 0 if wlo == 1 else (127 if whi == W - 2 else None)
                    v = np.zeros(10)
                    v[0] = 1.0
                    if dropA == 0: v[1] = -1.0
                    if dropA == 127: v[2] = -1.0
                    if dropB == 0: v[3] = -1.0
                    if dropB == 127: v[4] = -1.0
                    if dropA is not None and dropB is not None:
                        v[{(0, 0): 5, (0, 127): 6, (127, 0): 7, (127, 127): 8}[(dropA, dropB)]] = 1.0
                    for k in range(Kk):
                        cw[:, :, k] += w_net[k, 0, i, j, l] * scale * np.outer(cmask, v)
    btot = (net0_b + net1_b + net2_b).astype(np.float64)
    cw[:, 9, :] += btot[None, :] / C
    return cw


EDGE_CH = [0, 1, 62, 63]


def _make_cw2(cw):
    """Expand CW (C,10,K) to the NBASIS device basis columns:
    0 = PE selector column over elems [0,SMM_END) (partitions 0-3 =
        edge-channel partials, partition 4 = mid-channel grand partial),
    1=row0, 2=row127, 3..5=col0 parts, 6..8=col127 parts, 9..12=corners,
    13=const, 14.. = DVE per-channel total folds over FOLD_SPANS."""
    cwmid = cw[C // 2, 0, :]
    assert np.abs(cw[2:62, 0, :] - cwmid[None, :]).max() < 1e-12
    cwx = np.zeros((C, NBASIS, K), np.float64)
    for i, e in enumerate(EDGE_CH):
        cwx[i, 0, :] = cw[e, 0, :] - cwmid
    cwx[4, 0, :] = cwmid
    exp_map = [1, 2, 3, 3, 3, 4, 4, 4, 5, 6, 7, 8, 9]
    cwx[:, 1:14, :] = cw[:, exp_map, :]
    # the device col-part sums for rows [112,127) exclude row 127; its col
    # values are measured by the corner columns instead, so fold the
    # col-part coefficient into the corner coefficient
    cwx[:, 11, :] += cw[:, 3, :]
    cwx[:, 12, :] += cw[:, 4, :]
    cwx[:, 14:, :] = cw[:, [0] * (NBASIS - 14), :]
    return np.ascontiguousarray(cwx.astype(np.float32))


def _make_bank(Wt):
    """Wt (co, ci, 3, 3) -> (128, 6, 128): [p=(shift s, ci), m=(b,dw),
    (parity, co)].  Block [s][par] holds W[:, :, 1 + rowtap, 1 + dw].T where
    rowtap = (b + s) - par; invalid taps are zero."""
    bank = np.zeros((128, 6, 128), np.float32)
    for m, (b, dw) in enumerate(BCFG):
        for s in (0, 1):
            for par in (0, 1):
                rt = b + s - par
                if -1 <= rt <= 1:
                    bank[s * 64:s * 64 + 64, m, par * 64:par * 64 + 64] = \
                        Wt[:, :, 1 + rt, 1 + dw].T
    return bank


# ----------------------------------------------------------------------------
# device program
# ----------------------------------------------------------------------------
_NC_CACHE = {}


def _build_nc():
    import concourse.bacc as bacc
    import concourse.tile as tile
    from concourse import mybir

    f32 = mybir.dt.float32
    DT = mybir.dt.bfloat16
    Alu = mybir.AluOpType
    Ax = mybir.AxisListType
    Act = mybir.ActivationFunctionType

    F8 = mybir.dt.float8e3

    nc = bacc.Bacc("TRN2", target_bir_lowering=False, debug=False,
                   enable_asserts=False, num_devices=N)
    xin = nc.dram_tensor("xin", [C, H * WP], DT, kind="ExternalInput")
    xf8d = nc.dram_tensor("xf8", [C, H * WP], F8, kind="ExternalInput")
    seld = nc.dram_tensor("sel", [C, 128], F8, kind="ExternalInput")
    wbk = nc.dram_tensor("wbanks", [128, 6, K, 128], DT, kind="ExternalInput")
    cw2 = nc.dram_tensor("cw2", [C, NBASIS, K], f32, kind="ExternalInput")
    cb = nc.dram_tensor("convb", [128, 1], f32, kind="ExternalInput")
    # output partition p = parity*64 + channel; rows of one parity are
    # contiguous per partition so each eviction DMA is one 768B descriptor
    # per partition (host deinterleaves)
    outT = nc.dram_tensor("out", [128, NPAIR, W], DT, kind="ExternalOutput")

    with tile.TileContext(nc) as tc:
        with tc.tile_pool(name="singles", bufs=1) as S, \
             tc.tile_pool(name="stage", bufs=6) as STG, \
             tc.tile_pool(name="spsum", bufs=1, space="PSUM") as PS1, \
             tc.tile_pool(name="cpsum", bufs=NPSB, space="PSUM") as PS:

            XL = S.tile([128, NELEM], DT)
            XF = S.tile([C, H * WP], F8)
            wb_sb = S.tile([128, 6, K, 128], DT)
            cw2_sb = S.tile([C, NBASIS, K], f32)
            convb_sb = S.tile([128, 1], f32)
            zlhs = S.tile([128, 128], DT)
            sel = S.tile([64, 128], F8)
            onesall = S.tile([C, 128], DT)
            att_sb = S.tile([128, K], f32)
            M = S.tile([C, NBASIS], f32)
            G = S.tile([C, K], f32)
            Gb = S.tile([C, K], DT)
            gscr = S.tile([C, NBASIS], f32)
            mw = S.tile([128, 6, 128], DT)
            mwb = S.tile([128, 6, 128], DT)
            actscr = S.tile([C, 2080], f32)
            foldA = S.tile([C, 1104], DT)
            foldB = S.tile([C, 1104], DT)
            ssum = S.tile([128, 1], f32)
            sinv = S.tile([128, 1], f32)
            fence_sb = S.tile([C, 2], F8)

            psum_s = PS1.tile([128, SMM_W], f32)
            wpsum = PS1.tile([128, 512], f32)
            # logits land in a spare corner of the filler bank (frees a PSUM
            # bank for a 6th conv tile buffer)
            psum_b = wpsum[:, 480:480 + K]

            # --- constants / border zeroing (all tiny) ---
            nc.vector.memset(zlhs, 0.0)
            nc.vector.memset(onesall, 1.0)
            nc.vector.memset(M[:, 13:14], 1.0)
            # borders: host pre-pads the row gaps; only head/tail need zeroing
            nc.vector.memset(XL[0:64, 0:132], 0.0)
            nc.vector.memset(XL[0:64, 132 + H * WP:NELEM], 0.0)
            nc.vector.memset(XL[64:128, 0:2], 0.0)
            nc.vector.memset(XL[64:128, 2 + H * WP:NELEM], 0.0)

            # --- PE pipeline warm-up (results discarded; zlhs is all-zero) ---
            for i in range(8):
                nc.tensor.matmul(wpsum[:, 0:128], zlhs, zlhs, start=True, stop=True)

            # --- input DMAs, all on one ring so queue order is exact:
            # the small fp8 attention copy of x loads FIRST (it gates the
            # whole attention pipeline), then mixing banks and the bf16 conv
            # copy + its SBUF->SBUF row-shifted upper copies, interleaved in
            # conv-consumption order.  Output DMAs live on other rings.
            nc.scalar.dma_start(out=sel, in_=seld[:, :])
            nc.scalar.dma_start(out=cw2_sb, in_=cw2[:, :, :])
            nc.scalar.dma_start(out=convb_sb, in_=cb[:, :])
            # NOTE: descriptors of concurrently-pending DMAs round-robin
            # across the 16 queues, so trigger order alone does NOT make an
            # early chunk complete early.  Chunks that gate compute are
            # chained via a 1-element destination overlap (WAW dep) so they
            # truly complete in order.
            for c in range(len(XCHUNK_ROWS)):
                a = WP * XCHUNK_OFF[c]
                ln = WP * XCHUNK_ROWS[c]
                o = 1 if c > 0 else 0
                nc.sync.dma_start(out=XF[:, a - o:a + ln],
                                  in_=xf8d[:, a - o:a + ln])

            def bchunk(c):
                a = WP * BCHUNK_OFF[c]
                ln = WP * BCHUNK_ROWS[c]
                o = 1 if c > 0 else 0
                nc.sync.dma_start(out=XL[0:64, 132 + a - o: 132 + a + ln],
                                  in_=xin[:, a - o: a + ln])
                nc.sync.dma_start(out=XL[64:128, 2 + a: 2 + a + ln],
                                  in_=XL[0:64, 132 + a: 132 + a + ln])

            # fence: banks may not steal queue bandwidth from the fp8 stream
            nc.sync.dma_start(out=fence_sb, in_=XF[:, 16638:16640])
            nc.sync.dma_start(out=wb_sb[:, 0:2, :, :], in_=wbk[:, 0:2, :, :])
            bchunk(0)
            nc.sync.dma_start(out=wb_sb[:, 2:6, :, :], in_=wbk[:, 2:6, :, :])
            for c in range(1, len(BCHUNK_ROWS)):
                bchunk(c)

            # --- per-channel totals, split PE/DVE/ACT over the fp8 copy:
            # the PE accumulates selector matmuls over elems [0, SMM_END) as
            # chunks land (psum_s row i = edge-channel partial, row 4 = grand
            # partial); DVE and ACT fold the remaining spans per-channel.
            for j in range(NSMM):
                a = SMM_W * j
                nc.tensor.matmul(psum_s, sel, XF[:, a:a + SMM_W],
                                 start=(j == 0), stop=(j == NSMM - 1))

            def warm(n):
                # keep-warm fillers on data available early (results discarded)
                for i in range(n):
                    nc.tensor.matmul(wpsum[:, 0:512], sel, XF[:, 5120:5632],
                                     start=True, stop=True)

            # --- small basis sums (chunk-gated) ---
            def colpart(col, r0, r1, mcol):
                a = WP * r0 + col
                v = XF[:, a:a + WP * (r1 - r0)].rearrange(
                    "p (r w) -> p r w", w=WP)[:, :, 0:1]
                nc.vector.tensor_reduce(out=M[:, mcol:mcol + 1], in_=v,
                                        axis=Ax.XY, op=Alu.add)

            def fold(i, obuf):
                a, b = FOLD_SPANS[i]
                h = (b - a) // 2
                nc.vector.scalar_tensor_tensor(
                    out=obuf[:, :h], in0=XF[:, a:a + h], scalar=1.0,
                    in1=XF[:, a + h:b], op0=Alu.mult, op1=Alu.add,
                    accum_out=M[:, 14 + i:15 + i])

            # after chunk 0: row0 sum + row-0 corners
            nc.vector.tensor_reduce(out=M[:, 1:2], in_=XF[:, 0:W],
                                    axis=Ax.X, op=Alu.add)
            nc.vector.tensor_copy(
                out=M[:, 9:11].rearrange("p (a b) -> p a b", b=1),
                in_=XF[:, 0:254].rearrange("p (a b) -> p a b", b=127)[:, :, 0:1])
            colpart(0, 0, 64, 3)
            colpart(127, 0, 64, 6)
            fold(0, foldA)
            fold(1, foldB)
            colpart(0, 64, 112, 4)
            colpart(127, 64, 112, 7)
            fold(2, foldA)
            nc.vector.tensor_reduce(out=M[:, 2:3], in_=XF[:, 16510:16510 + W],
                                    axis=Ax.X, op=Alu.add)
            fold(3, foldB)
            # selector-PSUM reduce on the DVE right behind the last fold
            nc.vector.tensor_reduce(out=M[:, 0:1], in_=psum_s[0:64, :],
                                    axis=Ax.X, op=Alu.add)
            # ACT: mid-image spans + last-chunk smalls
            for i, (a, b) in enumerate(ACT_SPANS):
                nc.scalar.activation(out=actscr[:, 0:b - a], in_=XF[:, a:b],
                                     func=Act.Identity,
                                     accum_out=M[:, 18 + i:19 + i])
            # col parts rows [112,127); row 127's col values ride in the
            # corner basis columns (host folds the coefficients together)
            for col, mcol in ((0, 5), (127, 8)):
                a = WP * 112 + col
                v = XF[:, a:a + WP * 15].rearrange("p (r w) -> p r w", w=WP)[:, :, 0:1]
                nc.scalar.activation(out=actscr[:, 1040:1055].rearrange(
                    "p (r w) -> p r w", w=1), in_=v,
                    func=Act.Identity, accum_out=M[:, mcol:mcol + 1])
            nc.scalar.copy(out=M[:, 11:12], in_=XF[:, 16510:16511])
            nc.scalar.copy(out=M[:, 12:13], in_=XF[:, 16637:16638])

            # per-channel coefficient contraction: G[c,k] = sum_b M[c,b]*CW2[c,b,k]
            for k in range(K):
                nc.vector.scalar_tensor_tensor(
                    out=gscr, in0=M, scalar=1.0,
                    in1=cw2_sb[:, :, k], op0=Alu.mult, op1=Alu.mult,
                    accum_out=G[:, k:k + 1])

            # keep-warm fillers: PE issue is in-order, so the first batch
            # runs from selector end until the logits are ready; the second
            # batch bridges softmax+mixing into the conv.
            warm(3)
            # logits broadcast to all 128 partitions with one bf16 matmul
            nc.vector.tensor_copy(out=Gb, in_=G)
            nc.tensor.matmul(psum_b, onesall, Gb, start=True, stop=True)
            # unnormalized softmax: att = exp(logits); 1/sum applied at eviction
            nc.scalar.activation(out=att_sb, in_=psum_b, func=Act.Exp)
            warm(2)

            # --- weight mixing: mwb[:,m,:] = sum_k exp_k * bank'_k[:,m,:]
            # (all-bf16 so the DVE runs at its 16-bit 2x rate) ---
            def mixbank(m):
                nc.vector.tensor_scalar_mul(out=mw[:, m, :], in0=wb_sb[:, m, 0, :],
                                            scalar1=att_sb[:, 0:1])
                for k in range(1, K):
                    tgt = mwb if k == K - 1 else mw
                    nc.vector.scalar_tensor_tensor(
                        out=tgt[:, m, :], in0=wb_sb[:, m, k, :],
                        scalar=att_sb[:, k:k + 1], in1=mw[:, m, :],
                        op0=Alu.mult, op1=Alu.add)

            mixbank(0)
            mixbank(1)
            nc.vector.tensor_reduce(out=ssum, in_=att_sb, axis=Ax.X, op=Alu.add)
            nc.vector.reciprocal(out=sinv, in_=ssum)
            for m in range(2, 6):
                mixbank(m)

            # --- main conv (even/odd interleaved, 6 rows per tile) ---
            def tile_jt(t):
                return min(JT, NPAIR - JT * t)

            def mktile(t):
                return PS.tile([128, WP * JT], f32,
                               tag="cps", name=f"cps{t}")[:, :WP * tile_jt(t)]

            def conv_mm(t, pt, m):
                b, dw = BCFG[m]
                j0 = JT * t
                jt = tile_jt(t)
                s0 = 132 + b * WP + dw - 1 + 2 * WP * j0
                rhs = XL[:, s0:s0 + 2 * WP * jt].rearrange(
                    "p (j w) -> p j w", w=2 * WP)[:, :, 0:WP]
                nc.tensor.matmul(pt, mwb[:, m, :], rhs,
                                 start=(m == 0), stop=(m == 5))

            def evict(t, pt):
                jt = tile_jt(t)
                st = STG.tile([128, W * JT], DT, tag="stg", name=f"stg{t}")
                # strip the pad columns here (strided read of PSUM) so the
                # staging buffer and the output DMA are fully contiguous
                pv = pt.rearrange("p (j w) -> p j w", w=WP)[:, :, 1:1 + W]
                sv = st[:, :W * jt].rearrange("p (j w) -> p j w", w=W)
                if t % 2 == 0:
                    nc.scalar.activation(out=sv, in_=pv,
                                         func=Act.Identity,
                                         bias=convb_sb[:, 0:1],
                                         scale=sinv[:, 0:1])
                else:
                    nc.vector.tensor_scalar(
                        out=sv, in0=pv,
                        scalar1=sinv[:, 0:1],
                        scalar2=convb_sb[:, 0:1],
                        op0=Alu.mult, op1=Alu.add)
                eng = nc.scalar if t % 2 == 0 else nc.gpsimd
                j0 = JT * t
                eng.dma_start(out=outT[:, j0:j0 + jt, :], in_=sv)

            # head group m-outer: the first matmuls only need mixed bank 0
            pts = {t: mktile(t) for t in range(GS)}
            for m in range(6):
                for t in range(GS):
                    conv_mm(t, pts[t], m)
            for t in range(GS):
                evict(t, pts[t])
            # remaining tiles tile-major: evictions + output DMAs pipeline
            for t in range(GS, NT):
                pt = mktile(t)
                for m in range(6):
                    conv_mm(t, pt, m)
                evict(t, pt)

    nc.compile()
    return nc


def _get_nc():
    if "nc" not in _NC_CACHE:
        _NC_CACHE["nc"] = _build_nc()
    return _NC_CACHE["nc"]


def _prep_inputs(x, weight, conv_w, conv_b, net0_w, net0_b, net1_w, net1_b,
                 net2_w, net2_b):
    import ml_dtypes
    cw = _make_cw(np.asarray(net0_w, np.float32), np.asarray(net0_b, np.float32),
                  np.asarray(net1_w, np.float32), np.asarray(net1_b, np.float32),
                  np.asarray(net2_w, np.float32), np.asarray(net2_b, np.float32))
    cw2 = _make_cw2(cw)
    wf = np.asarray(weight, np.float32)
    cwf = np.asarray(conv_w, np.float32)
    # fold the static conv bank into every mixed bank (sum(att) == 1)
    banks = np.stack([_make_bank(wf[k] + cwf) for k in range(K)])  # (K,128,6,128)
    banks = np.ascontiguousarray(
        banks.transpose(1, 2, 0, 3)).astype(ml_dtypes.bfloat16)    # (128,6,K,128)
    convb = np.ascontiguousarray(
        np.tile(np.asarray(conv_b, np.float32), 2).reshape(128, 1))
    selh = np.zeros((C, 128), np.float32)
    for i, e in enumerate(EDGE_CH):
        selh[e, i] = 1.0
    selh[:, 4] = 1.0
    selh = np.ascontiguousarray(selh.astype(ml_dtypes.float8_e3m4))
    x = np.asarray(x, np.float32)
    xp = np.zeros((N, C, H, WP), np.float32)
    xp[:, :, :, :W] = x
    xs = xp.astype(ml_dtypes.bfloat16)
    xf = xp.astype(ml_dtypes.float8_e3m4)
    in_maps = []
    for n in range(N):
        in_maps.append({
            "xin": np.ascontiguousarray(xs[n].reshape(C, H * WP)),
            "xf8": np.ascontiguousarray(xf[n].reshape(C, H * WP)),
            "sel": selh,
            "wbanks": banks,
            "cw2": cw2,
            "convb": convb,
        })
    return in_maps


def _run(inputs, trace=False, **kw):
    from concourse.bass_utils import run_bass_kernel_spmd
    nc = _get_nc()
    in_maps = _prep_inputs(**inputs)
    return run_bass_kernel_spmd(nc, in_maps, core_ids=list(range(N)), trace=trace, **kw)


def _gather(res):
    # out is [parity*64+c, H/2, W] bf16 -> (C, H, W) with rows deinterleaved
    return np.stack([np.asarray(res.results[n]["out"])
                     .reshape(2, C, NPAIR, W).transpose(1, 2, 0, 3)
                     .reshape(C, H, W)
                     for n in range(N)]).astype(np.float32)


def kernel(**inputs):
    return _gather(_run(inputs))


# revision 48
# speedup vs baseline: 1.2344x; 1.2344x over previous
"""CondConv2d on 8 Trainium2 NeuronCores — data-parallel over batch N=8.

Per-core (one sample), all conv data in bf16:
  - x is read from HBM ONCE (lower partitions 0-63) in 8 chunks; the row-
    shifted upper copy (partitions 64-127) is produced by SBUF->SBUF DMAs
    whose descriptors drain behind the remaining loads — the conv consumes
    upper chunks far later than they arrive.
  - The attention branch (three global-mean-pooled conv3ds) collapses to a
    linear function of basis sums of x.  The expensive per-channel totals
    exploit that the conv3d depth masks only differentiate channels
    {0,1,62,63}: a selector matmul accumulates [4 edge-channel totals +
    grand total] into one PSUM bank on the otherwise-idle PE as chunks land,
    and one 512-wide DVE reduce + per-partition coefficients absorb them.
    Edge rows/cols/corners and the trailing total-folds are tiny chunk-gated
    DVE/ACT ops writing columns of one [64,19] basis matrix; a fused 4-op
    DVE contraction produces the logits.
  - Softmax normalization is skipped: weights are mixed with raw exp(logits)
    (the static conv_w bank is pre-folded into each bank host-side since
    sum(att)=1), and the 1/sum(exp) scale is applied at PSUM eviction
    together with the conv bias.
  - The 3x3 conv runs even/odd-row interleaved at 75% PE utilization:
    PSUM partitions = 64 out-ch x {even,odd} rows, contraction 128 =
    64 in-ch x {row r, row r+1} (the shifted upper copy).  6 matmuls
    (2 row-bases x 3 width taps) of [128x128x390] cover SIX output rows
    (3 row-pairs) per PSUM tile — 22 tiles total, half the PE cycles of a
    64-wide layout.  The rhs is a strided (pair, 130) view of XL; output
    rows deinterleave in the two eviction DMAs.  Output is stored bf16.
  - att-gated filler matmuls bridge the softmax/mixing window so the PE
    p-state stays ramped into the conv.
"""
import numpy as np

CONV_DT = "bf16"
N, C, H, W = 8, 64, 128, 128
K = 4
WP = W + 2                 # padded row width (130)
NELEM = WP * H + 262       # per-partition x buffer length (16902)
JT = 3                     # row-pairs per conv tile (6 output rows)
NPAIR = H // 2
GS = 5                     # conv tiles in the m-outer head group
NPSB = 6                   # conv PSUM tile buffers
NT = (NPAIR + JT - 1) // JT  # 22

# fp8 attention copy of x: staggered load chunks (concurrently-pending DMAs
# round-robin descriptors, so completion order comes from size asymmetry)
XCHUNK_ROWS = [12, 28, 36, 36, 16]
XCHUNK_OFF = list(np.cumsum([0] + XCHUNK_ROWS))
# bf16 conv copy of x: chunks sized to stay ahead of the conv wavefront
BCHUNK_ROWS = [24, 24, 32, 32, 16]
BCHUNK_OFF = list(np.cumsum([0] + BCHUNK_ROWS))

# conv matmul configs: (row base b, width tap dw)
BCFG = [(-1, -1), (-1, 0), (-1, 1), (1, -1), (1, 0), (1, 1)]

NBASIS = 20
SMM_W = 512                # selector-matmul free width
# selector segments: head while chunks land, tail after the last chunk
# (the PE runs them back-to-back into one PSUM accumulation group)
SMM_SEGS = [(0, 3072), (13824, 16384)]
# DVE fold spans for the rest of the totals (gating is automatic)
FOLD_SPANS = [(3072, 5200), (5200, 8320), (9880, 11904), (16384, 16640)]
# spans summed on the ACT engine (basis cols 18..19)
ACT_SPANS = [(8320, 9880), (11904, 13824)]


# ----------------------------------------------------------------------------
# host-side prep
# ----------------------------------------------------------------------------
def _make_cw(net0_w, net0_b, net1_w, net1_b, net2_w, net2_b):
    """CW[c, b, k] over the 10 logical bases:
    0=total, 1=row0, 2=row127, 3=col0, 4=col127,
    5..8=corners (00,0W,H0,HW), 9=const 1."""
    cw = np.zeros((C, 10, K), np.float64)
    scale = 1.0 / (C * H * W)
    for w_net, pads in ((net0_w, (0, 0, 0)), (net1_w, (1, 1, 1)), (net2_w, (2, 1, 1))):
        Kk, _, kd, kh, kw = w_net.shape
        pd, ph, pw = pads
        for i in range(kd):
            clo, chi = max(0, i - pd), min(C - 1, C - 1 + i - pd)
            cmask = np.zeros(C)
            cmask[clo:chi + 1] = 1.0
            for j in range(kh):
                hlo, hhi = max(0, j - ph), min(H - 1, H - 1 + j - ph)
                dropA = 0 if hlo == 1 else (127 if hhi == H - 2 else None)
                for l in range(kw):
                    wlo, whi = max(0, l - pw), min(W - 1, W - 1 + l - pw)
                    dropB = 0 if wlo == 1 else (127 if whi == W - 2 else None)
                    v = np.zeros(10)
                    v[0] = 1.0
                    if dropA == 0: v[1] = -1.0
                    if dropA == 127: v[2] = -1.0
                    if dropB == 0: v[3] = -1.0
                    if dropB == 127: v[4] = -1.0
                    if dropA is not None and dropB is not None:
                        v[{(0, 0): 5, (0, 127): 6, (127, 0): 7, (127, 127): 8}[(dropA, dropB)]] = 1.0
                    for k in range(Kk):
                        cw[:, :, k] += w_net[k, 0, i, j, l] * scale * np.outer(cmask, v)
    btot = (net0_b + net1_b + net2_b).astype(np.float64)
    cw[:, 9, :] += btot[None, :] / C
    return cw


EDGE_CH = [0, 1, 62, 63]


def _make_cw2(cw):
    """Expand CW (C,10,K) to the NBASIS device basis columns:
    0 = PE selector column over elems [0,SMM_END) (partitions 0-3 =
        edge-channel partials, partition 4 = mid-channel grand partial),
    1=row0, 2=row127, 3..5=col0 parts, 6..8=col127 parts, 9..12=corners,
    13=const, 14.. = DVE per-channel total folds over FOLD_SPANS."""
    cwmid = cw[C // 2, 0, :]
    assert np.abs(cw[2:62, 0, :] - cwmid[None, :]).max() < 1e-12
    cwx = np.zeros((C, NBASIS, K), np.float64)
    for i, e in enumerate(EDGE_CH):
        cwx[i, 0, :] = cw[e, 0, :] - cwmid
    cwx[4, 0, :] = cwmid
    exp_map = [1, 2, 3, 3, 3, 4, 4, 4, 5, 6, 7, 8, 9]
    cwx[:, 1:14, :] = cw[:, exp_map, :]
    # the device col-part sums for rows [112,127) exclude row 127; its col
    # values are measured by the corner columns instead, so fold the
    # col-part coefficient into the corner coefficient
    cwx[:, 11, :] += cw[:, 3, :]
    cwx[:, 12, :] += cw[:, 4, :]
    cwx[:, 14:, :] = cw[:, [0] * (NBASIS - 14), :]
    return np.ascontiguousarray(cwx.astype(np.float32))


def _make_bank(Wt):
    """Wt (co, ci, 3, 3) -> (128, 6, 128): [p=(shift s, ci), m=(b,dw),
    (parity, co)].  Block [s][par] holds W[:, :, 1 + rowtap, 1 + dw].T where
    rowtap = (b + s) - par; invalid taps are zero."""
    bank = np.zeros((128, 6, 128), np.float32)
    for m, (b, dw) in enumerate(BCFG):
        for s in (0, 1):
            for par in (0, 1):
                rt = b + s - par
                if -1 <= rt <= 1:
                    bank[s * 64:s * 64 + 64, m, par * 64:par * 64 + 64] = \
                        Wt[:, :, 1 + rt, 1 + dw].T
    return bank


# ----------------------------------------------------------------------------
# device program
# ----------------------------------------------------------------------------
_NC_CACHE = {}


def _build_nc():
    import concourse.bacc as bacc
    import concourse.tile as tile
    from concourse import mybir

    f32 = mybir.dt.float32
    DT = mybir.dt.bfloat16
    Alu = mybir.AluOpType
    Ax = mybir.AxisListType
    Act = mybir.ActivationFunctionType

    F8 = mybir.dt.float8e3

    nc = bacc.Bacc("TRN2", target_bir_lowering=False, debug=False,
                   enable_asserts=False, num_devices=N)
    xin = nc.dram_tensor("xin", [C, H * WP], DT, kind="ExternalInput")
    xf8d = nc.dram_tensor("xf8", [C, H * WP], F8, kind="ExternalInput")
    seld = nc.dram_tensor("sel", [C, 128], F8, kind="ExternalInput")
    wbk = nc.dram_tensor("wbanks", [128, 6, K, 128], DT, kind="ExternalInput")
    cw2 = nc.dram_tensor("cw2", [C, NBASIS, K], f32, kind="ExternalInput")
    cb = nc.dram_tensor("convb", [128, 1], f32, kind="ExternalInput")
    # output partition p = parity*64 + channel; rows of one parity are
    # contiguous per partition so each eviction DMA is one 768B descriptor
    # per partition (host deinterleaves)
    outT = nc.dram_tensor("out", [128, NPAIR, W], DT, kind="ExternalOutput")

    with tile.TileContext(nc) as tc:
        with tc.tile_pool(name="singles", bufs=1) as S, \
             tc.tile_pool(name="spsum", bufs=1, space="PSUM") as PS1, \
             tc.tile_pool(name="cpsum", bufs=NPSB, space="PSUM") as PS:

            XL = S.tile([128, NELEM], DT)
            XF = S.tile([C, H * WP], F8)
            wb_sb = S.tile([128, 6, K, 128], DT)
            cw2_sb = S.tile([C, NBASIS, K], f32)
            convb_sb = S.tile([128, 1], f32)
            zlhs = S.tile([128, 128], DT)
            sel = S.tile([64, 128], F8)
            onesall = S.tile([C, 128], DT)
            att_sb = S.tile([128, K], f32)
            M = S.tile([C, NBASIS], f32)
            G = S.tile([C, K], f32)
            Gb = S.tile([C, K], DT)
            gscr = S.tile([C, NBASIS], f32)
            mw = S.tile([128, 6, 128], DT)
            mwb = S.tile([128, 6, 128], DT)
            actscr = S.tile([C, 2080], f32)
            foldA = S.tile([C, 1560], DT)
            foldB = S.tile([C, 1560], DT)
            SG = S.tile([128, NPAIR * W], DT)
            ssum = S.tile([128, 1], f32)
            sinv = S.tile([128, 1], f32)

            psum_s = PS1.tile([128, SMM_W], f32)
            wpsum = PS1.tile([128, 512], f32)
            # logits land in a spare corner of the filler bank (frees a PSUM
            # bank for a 6th conv tile buffer)
            psum_b = wpsum[:, 480:480 + K]

            # --- constants / border zeroing (all tiny) ---
            nc.vector.memset(zlhs, 0.0)
            nc.vector.memset(onesall, 1.0)
            nc.vector.memset(M[:, 13:14], 1.0)
            # borders: host pre-pads the row gaps; only head/tail need zeroing
            nc.vector.memset(XL[0:64, 0:132], 0.0)
            nc.vector.memset(XL[0:64, 132 + H * WP:NELEM], 0.0)
            nc.vector.memset(XL[64:128, 0:2], 0.0)
            nc.vector.memset(XL[64:128, 2 + H * WP:NELEM], 0.0)

            # --- PE pipeline warm-up (results discarded; zlhs is all-zero) ---
            for i in range(8):
                nc.tensor.matmul(wpsum[:, 0:128], zlhs, zlhs, start=True, stop=True)

            # --- input DMAs, all on one ring so queue order is exact:
            # the small fp8 attention copy of x loads FIRST (it gates the
            # whole attention pipeline), then mixing banks and the bf16 conv
            # copy + its SBUF->SBUF row-shifted upper copies, interleaved in
            # conv-consumption order.  Output DMAs live on other rings.
            nc.scalar.dma_start(out=sel, in_=seld[:, :])
            nc.scalar.dma_start(out=cw2_sb, in_=cw2[:, :, :])
            nc.scalar.dma_start(out=convb_sb, in_=cb[:, :])
            for c in range(len(XCHUNK_ROWS)):
                a = WP * XCHUNK_OFF[c]
                ln = WP * XCHUNK_ROWS[c]
                nc.sync.dma_start(out=XF[:, a:a + ln], in_=xf8d[:, a:a + ln])

            def bchunk(c):
                a = WP * BCHUNK_OFF[c]
                ln = WP * BCHUNK_ROWS[c]
                nc.sync.dma_start(out=XL[0:64, 132 + a: 132 + a + ln],
                                  in_=xin[:, a: a + ln])
                nc.sync.dma_start(out=XL[64:128, 2 + a: 2 + a + ln],
                                  in_=XL[0:64, 132 + a: 132 + a + ln])

            nc.sync.dma_start(out=wb_sb[:, 0:2, :, :], in_=wbk[:, 0:2, :, :])
            bchunk(0)
            nc.sync.dma_start(out=wb_sb[:, 2:6, :, :], in_=wbk[:, 2:6, :, :])
            for c in range(1, len(BCHUNK_ROWS)):
                bchunk(c)

            # --- per-channel totals, split PE/DVE/ACT over the fp8 copy:
            # the PE accumulates selector matmuls over the SMM_SEGS spans as
            # chunks land (psum_s row i = edge-channel partial, row 4 = grand
            # partial); DVE and ACT fold the remaining spans per-channel.
            segs = [(a + SMM_W * j) for a, b in SMM_SEGS
                    for j in range((b - a) // SMM_W)]
            for j, a in enumerate(segs):
                nc.tensor.matmul(psum_s, sel, XF[:, a:a + SMM_W],
                                 start=(j == 0), stop=(j == len(segs) - 1))

            def warm(n):
                # keep-warm fillers on data available early (results discarded)
                for i in range(n):
                    nc.tensor.matmul(wpsum[:, 0:512], sel, XF[:, 5120:5632],
                                     start=True, stop=True)

            # --- small basis sums (chunk-gated) ---
            def colpart(col, r0, r1, mcol):
                a = WP * r0 + col
                v = XF[:, a:a + WP * (r1 - r0)].rearrange(
                    "p (r w) -> p r w", w=WP)[:, :, 0:1]
                nc.vector.tensor_reduce(out=M[:, mcol:mcol + 1], in_=v,
                                        axis=Ax.XY, op=Alu.add)

            def fold(i, obuf):
                a, b = FOLD_SPANS[i]
                h = (b - a) // 2
                nc.vector.scalar_tensor_tensor(
                    out=obuf[:, :h], in0=XF[:, a:a + h], scalar=1.0,
                    in1=XF[:, a + h:b], op0=Alu.mult, op1=Alu.add,
                    accum_out=M[:, 14 + i:15 + i])

            # after chunk 0: row0 sum + row-0 corners
            nc.vector.tensor_reduce(out=M[:, 1:2], in_=XF[:, 0:W],
                                    axis=Ax.X, op=Alu.add)
            nc.vector.tensor_copy(
                out=M[:, 9:11].rearrange("p (a b) -> p a b", b=1),
                in_=XF[:, 0:254].rearrange("p (a b) -> p a b", b=127)[:, :, 0:1])
            colpart(0, 0, 64, 3)
            colpart(127, 0, 64, 6)
            fold(0, foldA)
            fold(1, foldB)
            colpart(0, 64, 112, 4)
            colpart(127, 64, 112, 7)
            fold(2, foldA)
            nc.vector.tensor_reduce(out=M[:, 2:3], in_=XF[:, 16510:16510 + W],
                                    axis=Ax.X, op=Alu.add)
            fold(3, foldB)
            # ACT: mid-image spans, last-chunk smalls, selector-PSUM reduce
            for i, (a, b) in enumerate(ACT_SPANS):
                nc.scalar.activation(out=actscr[:, 0:b - a], in_=XF[:, a:b],
                                     func=Act.Identity,
                                     accum_out=M[:, 18 + i:19 + i])
            # col parts rows [112,127); row 127's col values ride in the
            # corner basis columns (host folds the coefficients together)
            for col, mcol in ((0, 5), (127, 8)):
                a = WP * 112 + col
                v = XF[:, a:a + WP * 15].rearrange("p (r w) -> p r w", w=WP)[:, :, 0:1]
                nc.scalar.activation(out=actscr[:, 1040:1055].rearrange(
                    "p (r w) -> p r w", w=1), in_=v,
                    func=Act.Identity, accum_out=M[:, mcol:mcol + 1])
            nc.scalar.copy(out=M[:, 11:12], in_=XF[:, 16510:16511])
            nc.scalar.copy(out=M[:, 12:13], in_=XF[:, 16637:16638])
            nc.scalar.activation(out=actscr[:, 0:SMM_W], in_=psum_s[0:64, :],
                                 func=Act.Identity, accum_out=M[:, 0:1])

            # per-channel coefficient contraction: G[c,k] = sum_b M[c,b]*CW2[c,b,k]
            for k in range(K):
                nc.vector.scalar_tensor_tensor(
                    out=gscr, in0=M, scalar=1.0,
                    in1=cw2_sb[:, :, k], op0=Alu.mult, op1=Alu.mult,
                    accum_out=G[:, k:k + 1])

            # keep-warm fillers: PE issue is in-order, so the first batch
            # runs from selector end until the logits are ready; the second
            # batch bridges softmax+mixing into the conv.
            warm(6)
            # logits broadcast to all 128 partitions with one bf16 matmul
            nc.vector.tensor_copy(out=Gb, in_=G)
            nc.tensor.matmul(psum_b, onesall, Gb, start=True, stop=True)
            # unnormalized softmax: att = exp(logits); 1/sum applied at eviction
            nc.scalar.activation(out=att_sb, in_=psum_b, func=Act.Exp)
            warm(2)

            # --- weight mixing: mwb[:,m,:] = sum_k exp_k * bank'_k[:,m,:]
            # (all-bf16 so the DVE runs at its 16-bit 2x rate) ---
            def mixbank(m):
                nc.vector.tensor_scalar_mul(out=mw[:, m, :], in0=wb_sb[:, m, 0, :],
                                            scalar1=att_sb[:, 0:1])
                for k in range(1, K):
                    tgt = mwb if k == K - 1 else mw
                    nc.vector.scalar_tensor_tensor(
                        out=tgt[:, m, :], in0=wb_sb[:, m, k, :],
                        scalar=att_sb[:, k:k + 1], in1=mw[:, m, :],
                        op0=Alu.mult, op1=Alu.add)

            mixbank(0)
            mixbank(1)
            nc.vector.tensor_reduce(out=ssum, in_=att_sb, axis=Ax.X, op=Alu.add)
            nc.vector.reciprocal(out=sinv, in_=ssum)
            for m in range(2, 6):
                mixbank(m)

            # --- main conv (even/odd interleaved, 6 rows per tile) ---
            def tile_jt(t):
                return min(JT, NPAIR - JT * t)

            def mktile(t):
                return PS.tile([128, WP * JT], f32,
                               tag="cps", name=f"cps{t}")[:, :WP * tile_jt(t)]

            def conv_mm(t, pt, m):
                b, dw = BCFG[m]
                j0 = JT * t
                jt = tile_jt(t)
                s0 = 132 + b * WP + dw - 1 + 2 * WP * j0
                rhs = XL[:, s0:s0 + 2 * WP * jt].rearrange(
                    "p (j w) -> p j w", w=2 * WP)[:, :, 0:WP]
                nc.tensor.matmul(pt, mwb[:, m, :], rhs,
                                 start=(m == 0), stop=(m == 5))

            def evict(t, pt):
                jt = tile_jt(t)
                j0 = JT * t
                # strip the pad columns here (strided read of PSUM) into one
                # big contiguous staging buffer; DMAs go out in 4-tile
                # groups so each descriptor is a 3KB contiguous run
                pv = pt.rearrange("p (j w) -> p j w", w=WP)[:, :, 1:1 + W]
                sv = SG[:, W * j0:W * (j0 + jt)].rearrange(
                    "p (j w) -> p j w", w=W)
                if t % 2 == 0:
                    nc.scalar.activation(out=sv, in_=pv,
                                         func=Act.Identity,
                                         bias=convb_sb[:, 0:1],
                                         scale=sinv[:, 0:1])
                else:
                    nc.vector.tensor_scalar(
                        out=sv, in0=pv,
                        scalar1=sinv[:, 0:1],
                        scalar2=convb_sb[:, 0:1],
                        op0=Alu.mult, op1=Alu.add)
                if t in (3, 7, 11, 15, 19, NT - 1):
                    p0 = JT * (t - 3) if t != NT - 1 else JT * 20
                    p1 = JT * t + jt
                    eng = nc.scalar if (t // 4) % 2 == 0 else nc.gpsimd
                    eng.dma_start(
                        out=outT[:, p0:p1, :],
                        in_=SG[:, W * p0:W * p1].rearrange("p (j w) -> p j w", w=W))

            # head group m-outer: the first matmuls only need mixed bank 0
            pts = {t: mktile(t) for t in range(GS)}
            for m in range(6):
                for t in range(GS):
                    conv_mm(t, pts[t], m)
            for t in range(GS):
                evict(t, pts[t])
            # remaining tiles tile-major: evictions + output DMAs pipeline
            for t in range(GS, NT):
                pt = mktile(t)
                for m in range(6):
                    conv_mm(t, pt, m)
                evict(t, pt)

    nc.compile()
    return nc


def _get_nc():
    if "nc" not in _NC_CACHE:
        _NC_CACHE["nc"] = _build_nc()
    return _NC_CACHE["nc"]


def _prep_inputs(x, weight, conv_w, conv_b, net0_w, net0_b, net1_w, net1_b,
                 net2_w, net2_b):
    import ml_dtypes
    cw = _make_cw(np.asarray(net0_w, np.float32), np.asarray(net0_b, np.float32),
                  np.asarray(net1_w, np.float32), np.asarray(net1_b, np.float32),
                  np.asarray(net2_w, np.float32), np.asarray(net2_b, np.float32))
    cw2 = _make_cw2(cw)
    wf = np.asarray(weight, np.float32)
    cwf = np.asarray(conv_w, np.float32)
    # fold the static conv bank into every mixed bank (sum(att) == 1)
    banks = np.stack([_make_bank(wf[k] + cwf) for k in range(K)])  # (K,128,6,128)
    banks = np.ascontiguousarray(
        banks.transpose(1, 2, 0, 3)).astype(ml_dtypes.bfloat16)    # (128,6,K,128)
    convb = np.ascontiguousarray(
        np.tile(np.asarray(conv_b, np.float32), 2).reshape(128, 1))
    selh = np.zeros((C, 128), np.float32)
    for i, e in enumerate(EDGE_CH):
        selh[e, i] = 1.0
    selh[:, 4] = 1.0
    selh = np.ascontiguousarray(selh.astype(ml_dtypes.float8_e3m4))
    x = np.asarray(x, np.float32)
    xp = np.zeros((N, C, H, WP), np.float32)
    xp[:, :, :, :W] = x
    xs = xp.astype(ml_dtypes.bfloat16)
    xf = xp.astype(ml_dtypes.float8_e3m4)
    in_maps = []
    for n in range(N):
        in_maps.append({
            "xin": np.ascontiguousarray(xs[n].reshape(C, H * WP)),
            "xf8": np.ascontiguousarray(xf[n].reshape(C, H * WP)),
            "sel": selh,
            "wbanks": banks,
            "cw2": cw2,
            "convb": convb,
        })
    return in_maps


def _run(inputs, trace=False, **kw):
    from concourse.bass_utils import run_bass_kernel_spmd
    nc = _get_nc()
    in_maps = _prep_inputs(**inputs)
    return run_bass_kernel_spmd(nc, in_maps, core_ids=list(range(N)), trace=trace, **kw)


def _gather(res):
    # out is [parity*64+c, H/2, W] bf16 -> (C, H, W) with rows deinterleaved
    return np.stack([np.asarray(res.results[n]["out"])
                     .reshape(2, C, NPAIR, W).transpose(1, 2, 0, 3)
                     .reshape(C, H, W)
                     for n in range(N)]).astype(np.float32)


def kernel(**inputs):
    return _gather(_run(inputs))
